# revision 1
# baseline (speedup 1.0000x reference)
"""Trainium2 Bass kernel for nn_Pixelwise_77919296684103.

Depth decode via structured two-harmonic model:
  BVals standardization + nearest-neighbor over a 10000-entry code table
  collapses to argmin over theta of S(t) = U cos2t + V sin2t + P cost + Q sint
  (per pixel), solved with a 128-point coarse grid argmin (PE matmul) + 3
  clamped Newton steps. cos/sin computed on-device by quadrant-reduced
  Taylor polynomials (no gathers needed).

Sharding: data-parallel over pixels. 19200 pixels -> 8 cores x 2400
(padded to 2432 = 128x19 tiles). Mod/Demod tables replicated per core.
"""
import numpy as np
import sys

for _p in ("/opt/trn_rl_repo",):
    if _p not in sys.path:
        sys.path.insert(0, _p)

from concourse import bass, mybir
import concourse.tile as tile_mod
import concourse.bass2jax as _b2j
from concourse.vector_clock import ScopedClock
from concourse.masks import make_identity
from concourse.bass_utils import run_bass_kernel_spmd

# ---------------------------------------------------------------------------
# Patches: this walrus build allows only ONE semaphore wait per instruction.
# 1) TileContext exit Drain: split its sem waits across NOPs.
# 2) Global BIR pass: hoist extra waits onto NoOps before the owner.
# ---------------------------------------------------------------------------
if not getattr(tile_mod, "_onewait_patched", False):
    tile_mod._onewait_patched = True

    def _patched_drain_and_barrier(self, tick_clock, wait_clock):
        nc = self.nc
        probe = nc.sync.nop(nofuse=True)
        wait_clock.add_sem_waits(probe.ins, ScopedClock({None: tick_clock.global_clock}))
        si = probe.ins.sync_info
        waits = list(si.on_wait) if si is not None else []
        if len(waits) > 1:
            si.on_wait = waits[:1]
            for w in waits[1:]:
                nop = nc.sync.nop(nofuse=True)
                nop.ins.sync_info = mybir.SyncInfo(on_wait=[w], on_update=[])
        nc.sync.drain()
        nc.all_engine_barrier()
        assert self.sems is not None
        popped = nc._tile_sem_poison_stack.pop()
        assert popped is self._sem_poison
        nc.clear_and_free_semaphores(list(self.sems.allocated().values()))
        nc.all_engine_barrier()

    tile_mod.TileContext._drain_and_barrier = _patched_drain_and_barrier

    import json as _json

    _orig_decompress = _b2j._decompress_ant_bir

    def _fix_bir_bytes(raw: bytes) -> bytes:
        bir = _json.loads(raw)
        changed = False
        for fn in bir.get("functions", []):
            for bb in fn.get("blocks", []):
                newlist = []
                for ins in bb.get("instructions", []):
                    si = ins.get("sync_info")
                    waits = (si or {}).get("on_wait") or []
                    if len(waits) > 1:
                        changed = True
                        for j, wx in enumerate(waits[:-1]):
                            newlist.append({
                                "debug": ins.get("debug"),
                                "engine": ins["engine"],
                                "ins": [],
                                "name": f"{ins['name']}w{j}",
                                "opcode": "NoOp",
                                "outs": [],
                                "sync_info": {"on_update": [], "on_wait": [wx]},
                            })
                        si["on_wait"] = waits[-1:]
                    newlist.append(ins)
                bb["instructions"] = newlist
        if not changed:
            return raw
        return _json.dumps(bir).encode()

    def _decompress_and_fix(data):
        return _fix_bir_bytes(_orig_decompress(data))

    _b2j._decompress_ant_bir = _decompress_and_fix

# ---------------------------------------------------------------------------
# Constants
# ---------------------------------------------------------------------------
f32 = mybir.dt.float32
i32 = mybir.dt.int32
u32 = mybir.dt.uint32
AX = mybir.AxisListType
OP = mybir.AluOpType
AF = mybir.ActivationFunctionType

nf32 = np.float32
N = 10000
K = 3
G = 128
NCORES = 8
PIX = 19200
PPC = 2400              # pixels per core
NT = 19                 # tiles of 128 per core (2432 padded)
C_LIGHT = 299792458.0 * 1000.0
TAU_MIN = 2.0 * 10000.0 / C_LIGHT
DT = float(nf32(TAU_MIN / N))
PA = float(nf32(1e6))
INV_N = float(nf32(1.0 / N))
CHAT2 = nf32(2.0 * (N - 1) / N)
CHAT = np.sqrt(CHAT2)                       # f32
HU = float(nf32(0.5) * CHAT2)               # U scale
HV = float(nf32(-0.5) * CHAT2)              # V scale
HP = float(nf32(-2.0) * CHAT)               # P scale
HPn = float(-(nf32(-2.0) * CHAT))           # -P
HQ = float(nf32(2.0) * CHAT)                # Q scale
HQn = float(-(nf32(2.0) * CHAT))            # -Q
SPACING = float(nf32(2.0 * np.pi / G))
NFC = float(nf32(N / (2.0 * np.pi)))
HPI = float(nf32(np.pi / 2.0))
C2, C4, C6 = float(nf32(-0.5)), float(nf32(1.0 / 24)), float(nf32(-1.0 / 720))
S3, S5, S7 = float(nf32(-1.0 / 6)), float(nf32(1.0 / 120)), float(nf32(-1.0 / 5040))
SC_UP = float(nf32(2.0 ** 56))              # exact pow2 prescale for sqrt
SC_DN = float(nf32(2.0 ** -28))


# ---------------------------------------------------------------------------
# Device program
# ---------------------------------------------------------------------------
def _emit_floor(nc, sb, shape, out_f, in_f):
    """out_f = floor(in_f), rounding-mode agnostic (int cast + is_gt fix)."""
    ii = sb.tile(shape, dtype=i32)
    nc.vector.tensor_copy(ii[:], in_f)
    nc.vector.tensor_copy(out_f, ii[:])
    mk = sb.tile(shape, dtype=f32)
    nc.vector.tensor_tensor(mk[:], out_f, in_f, OP.is_gt)
    nc.vector.tensor_tensor(out_f, out_f, mk[:], OP.subtract)


def _emit_cossin(nc, sb, shape, u4, cr, sr):
    """(cr, sr) = (cos, sin)(2*pi*t) where u4 = 4*t, t in [0,1).
    Quadrant reduction around nearest quarter + Taylor on [-pi/4, pi/4]."""
    v = sb.tile(shape, dtype=f32)
    nc.vector.tensor_scalar_add(v[:], u4, 0.5)
    q = sb.tile(shape, dtype=f32)
    _emit_floor(nc, sb, shape, q[:], v[:])
    phi = sb.tile(shape, dtype=f32)
    nc.vector.tensor_tensor(phi[:], u4, q[:], OP.subtract)
    nc.vector.tensor_scalar_mul(phi[:], phi[:], HPI)
    x = sb.tile(shape, dtype=f32)
    nc.vector.tensor_tensor(x[:], phi[:], phi[:], OP.mult)
    c = sb.tile(shape, dtype=f32)
    nc.vector.tensor_scalar(c[:], x[:], C6, C4, OP.mult, OP.add)
    nc.vector.tensor_tensor(c[:], c[:], x[:], OP.mult)
    nc.vector.tensor_scalar_add(c[:], c[:], C2)
    nc.vector.tensor_tensor(c[:], c[:], x[:], OP.mult)
    nc.vector.tensor_scalar_add(c[:], c[:], 1.0)
    s = sb.tile(shape, dtype=f32)
    nc.vector.tensor_scalar(s[:], x[:], S7, S5, OP.mult, OP.add)
    nc.vector.tensor_tensor(s[:], s[:], x[:], OP.mult)
    nc.vector.tensor_scalar_add(s[:], s[:], S3)
    nc.vector.tensor_tensor(s[:], s[:], x[:], OP.mult)
    nc.vector.tensor_scalar_add(s[:], s[:], 1.0)
    nc.vector.tensor_tensor(s[:], s[:], phi[:], OP.mult)
    # quadrant signs: sgc = [q==0]+[q==4]-[q==2], sgs = [q==1]-[q==3]
    e0 = sb.tile(shape, dtype=f32)
    nc.vector.tensor_scalar(e0[:], q[:], 0.0, None, OP.is_equal)
    e4 = sb.tile(shape, dtype=f32)
    nc.vector.tensor_scalar(e4[:], q[:], 4.0, None, OP.is_equal)
    e2 = sb.tile(shape, dtype=f32)
    nc.vector.tensor_scalar(e2[:], q[:], 2.0, None, OP.is_equal)
    sgc = sb.tile(shape, dtype=f32)
    nc.vector.tensor_tensor(sgc[:], e0[:], e4[:], OP.add)
    nc.vector.tensor_tensor(sgc[:], sgc[:], e2[:], OP.subtract)
    e1 = sb.tile(shape, dtype=f32)
    nc.vector.tensor_scalar(e1[:], q[:], 1.0, None, OP.is_equal)
    e3 = sb.tile(shape, dtype=f32)
    nc.vector.tensor_scalar(e3[:], q[:], 3.0, None, OP.is_equal)
    sgs = sb.tile(shape, dtype=f32)
    nc.vector.tensor_tensor(sgs[:], e1[:], e3[:], OP.subtract)
    ta = sb.tile(shape, dtype=f32)
    tg2 = sb.tile(shape, dtype=f32)
    nc.vector.tensor_tensor(cr, sgc[:], c[:], OP.mult)
    nc.vector.tensor_tensor(ta[:], sgs[:], s[:], OP.mult)
    nc.vector.tensor_tensor(cr, cr, ta[:], OP.subtract)
    nc.vector.tensor_tensor(sr, sgs[:], c[:], OP.mult)
    nc.vector.tensor_tensor(tg2[:], sgc[:], s[:], OP.mult)
    nc.vector.tensor_tensor(sr, sr, tg2[:], OP.add)


def _build():
    nc = bass.Bass()
    GIN = nc.dram_tensor("GIN", [128, NT], f32, kind="ExternalInput")
    MODR = nc.dram_tensor("MODR", [125, 240], f32, kind="ExternalInput")
    DEMR = nc.dram_tensor("DEMR", [125, 240], f32, kind="ExternalInput")
    CBD = nc.dram_tensor("CBD", [125, 240], f32, kind="ExternalInput")
    SBD = nc.dram_tensor("SBD", [125, 240], f32, kind="ExternalInput")
    ONES125 = nc.dram_tensor("ONES125", [125, 1], f32, kind="ExternalInput")
    ONES1 = nc.dram_tensor("ONES1", [1, 128], f32, kind="ExternalInput")
    GRIDC = nc.dram_tensor("GRIDC", [2, 128], f32, kind="ExternalInput")
    C2G = nc.dram_tensor("C2G", [1, 128], f32, kind="ExternalInput")
    S2G = nc.dram_tensor("S2G", [1, 128], f32, kind="ExternalInput")
    OUT = nc.dram_tensor("OUT", [128, NT], f32, kind="ExternalOutput")
    SCO = nc.dram_tensor("SCO", [1, 20], f32, kind="ExternalOutput")

    with tile_mod.TileContext(nc) as tc:
        with tc.tile_pool(name="sb", bufs=1) as sb, \
             tc.tile_pool(name="ps", bufs=1, space="PSUM") as ps, \
             tc.tile_pool(name="ps2", bufs=2, space="PSUM") as ps2:
            P19 = [128, NT]
            tt = nc.vector.tensor_tensor
            ts = nc.vector.tensor_scalar
            tsa = nc.vector.tensor_scalar_add
            tsm = nc.vector.tensor_scalar_mul

            # ---- input DMAs (GIN first: the pixel-path front depends only
            # on it, so vector can start while the tables stream in) ----
            gin = sb.tile(P19, dtype=f32)
            nc.gpsimd.dma_start(out=gin[:], in_=GIN[:])
            modr = sb.tile([125, 240], dtype=f32)
            nc.gpsimd.dma_start(out=modr[:], in_=MODR[:])
            demr = sb.tile([125, 240], dtype=f32)
            nc.gpsimd.dma_start(out=demr[:], in_=DEMR[:])
            cbt = sb.tile([125, 240], dtype=f32)
            nc.gpsimd.dma_start(out=cbt[:], in_=CBD[:])
            sbt = sb.tile([125, 240], dtype=f32)
            nc.gpsimd.dma_start(out=sbt[:], in_=SBD[:])
            o125 = sb.tile([125, 1], dtype=f32)
            nc.gpsimd.dma_start(out=o125[:], in_=ONES125[:])
            o1 = sb.tile([1, 128], dtype=f32)
            nc.gpsimd.dma_start(out=o1[:], in_=ONES1[:])
            # row order [W2n; cg; sg] so L3 row0 (the all-ones row) can be
            # memset on partition 0 (compute partition offsets must be 0 mod 32)
            grid = sb.tile([3, 128], dtype=f32)
            nc.gpsimd.dma_start(out=grid[1:3, :], in_=GRIDC[:])
            c2gt = sb.tile([1, 128], dtype=f32)
            nc.gpsimd.dma_start(out=c2gt[:], in_=C2G[:])
            s2gt = sb.tile([1, 128], dtype=f32)
            nc.gpsimd.dma_start(out=s2gt[:], in_=S2G[:])

            # ---- stage E front (hoisted): idx/cos/sin need only GIN ----
            uu = sb.tile(P19, dtype=f32)
            tsa(uu[:], gin[:], 0.5)
            fb = sb.tile(P19, dtype=f32)
            _emit_floor(nc, sb, P19, fb[:], uu[:])
            ts(fb[:], fb[:], 0.0, float(N - 1), OP.max, OP.min)
            u4 = sb.tile(P19, dtype=f32)
            tsm(u4[:], fb[:], 4.0 * INV_N)    # x4 exact, bit-identical
            ci = sb.tile(P19, dtype=f32)
            si = sb.tile(P19, dtype=f32)
            _emit_cossin(nc, sb, P19, u4[:], ci[:], si[:])

            # ---- stage A: harmonics 0 and 1 of Mod/Dem columns ----
            # RH[:, 6 blocks of 3] = per-partition partial sums over c (80)
            RH = sb.tile([125, 18], dtype=f32)

            def red3(out_sl, in_t):
                nc.vector.tensor_reduce(
                    out=out_sl, in_=in_t.rearrange("p (c k) -> p k c", k=3),
                    axis=AX.X, op=OP.add)

            red3(RH[:, 0:3], modr[:])                    # Ms0 partials
            tmpA = sb.tile([125, 240], dtype=f32)
            tt(tmpA[:], cbt[:], modr[:], OP.mult)
            red3(RH[:, 3:6], tmpA[:])                    # Mc1
            tmpB = sb.tile([125, 240], dtype=f32)
            tt(tmpB[:], sbt[:], modr[:], OP.mult)
            red3(RH[:, 6:9], tmpB[:])                    # Msn1
            red3(RH[:, 9:12], demr[:])                   # Ds0
            tmpC = sb.tile([125, 240], dtype=f32)
            tt(tmpC[:], cbt[:], demr[:], OP.mult)
            red3(RH[:, 12:15], tmpC[:])                  # Dc1
            tmpD = sb.tile([125, 240], dtype=f32)
            tt(tmpD[:], sbt[:], demr[:], OP.mult)
            red3(RH[:, 15:18], tmpD[:])                  # Dsn1

            ps18 = ps.tile([1, 18], dtype=f32)
            nc.tensor.matmul(ps18[:], o125[:], RH[:], start=True, stop=True)
            H = sb.tile([1, 18], dtype=f32)
            nc.vector.tensor_copy(H[:], ps18[:])
            Ms0, Mc1, Msn1 = H[:, 0:3], H[:, 3:6], H[:, 6:9]
            Ds0, Dc1, Dsn1 = H[:, 9:12], H[:, 12:15], H[:, 15:18]

            # ---- stage B: scalar stage on partition 0 ----
            # SC cols: a(0:3) Ck(3:6) Sk(6:9) amb(9:12) cw(12:15) sw(15:18) U(18) V(19)
            SC = sb.tile([1, 20], dtype=f32)
            t1 = sb.tile([1, 3], dtype=f32)
            t2 = sb.tile([1, 3], dtype=f32)
            # a_k = ((Ms0*Ds0)*(1/N))*DT
            tt(SC[:, 0:3], Ms0, Ds0, OP.mult)
            tsm(SC[:, 0:3], SC[:, 0:3], INV_N)
            tsm(SC[:, 0:3], SC[:, 0:3], DT)
            # Re1 = ((Mc1*Dc1 + Msn1*Dsn1)*(1/N))*DT ; Ck = 2*Re1
            tt(t1[:], Mc1, Dc1, OP.mult)
            tt(t2[:], Msn1, Dsn1, OP.mult)
            tt(t1[:], t1[:], t2[:], OP.add)
            tsm(t1[:], t1[:], INV_N)
            tsm(t1[:], t1[:], DT)
            tsm(SC[:, 3:6], t1[:], 2.0)
            # Im1 = ((Msn1*Dc1 - Mc1*Dsn1)*(1/N))*DT ; Sk = -2*Im1
            tt(t1[:], Msn1, Dc1, OP.mult)
            tt(t2[:], Mc1, Dsn1, OP.mult)
            tt(t1[:], t1[:], t2[:], OP.subtract)
            tsm(t1[:], t1[:], INV_N)
            tsm(t1[:], t1[:], DT)
            tsm(SC[:, 6:9], t1[:], -2.0)
            Ck, Sk = SC[:, 3:6], SC[:, 6:9]
            # r = sqrt(Ck^2+Sk^2) (pow2 prescaled); cw = Ck/r ; sw = -Sk/r
            tt(t1[:], Ck, Ck, OP.mult)
            tt(t2[:], Sk, Sk, OP.mult)
            tt(t1[:], t1[:], t2[:], OP.add)
            tsm(t1[:], t1[:], SC_UP)
            rr = sb.tile([1, 3], dtype=f32)
            nc.scalar.activation(rr[:], t1[:], AF.Sqrt)
            tsm(rr[:], rr[:], SC_DN)
            ri = sb.tile([1, 3], dtype=f32)
            nc.vector.reciprocal(ri[:], rr[:])
            tt(SC[:, 12:15], Ck, ri[:], OP.mult)
            tsm(t2[:], Sk, -1.0)
            tt(SC[:, 15:18], t2[:], ri[:], OP.mult)
            cw, sw = SC[:, 12:15], SC[:, 15:18]
            # U = HU*sum(cw^2-sw^2) ; V = HV*sum(2*cw*sw)
            tt(t1[:], cw, cw, OP.mult)
            tt(t2[:], sw, sw, OP.mult)
            tt(t1[:], t1[:], t2[:], OP.subtract)
            su = sb.tile([1, 1], dtype=f32)
            nc.vector.tensor_reduce(out=su[:], in_=t1[:], axis=AX.X, op=OP.add)
            tsm(SC[:, 18:19], su[:], HU)
            tsm(t1[:], cw, 2.0)
            tt(t1[:], t1[:], sw, OP.mult)
            sv = sb.tile([1, 1], dtype=f32)
            nc.vector.tensor_reduce(out=sv[:], in_=t1[:], axis=AX.X, op=OP.add)
            tsm(SC[:, 19:20], sv[:], HV)
            # amb = PA*(Ds0*DT)
            tsm(t2[:], Ds0, DT)
            tsm(SC[:, 9:12], t2[:], PA)
            nc.gpsimd.dma_start(out=SCO[:], in_=SC[:])

            # ---- stage C: broadcast SC to all 128 partitions ----
            psb = ps.tile([128, 20], dtype=f32)
            nc.tensor.matmul(psb[:], o1[:], SC[:], start=True, stop=True)
            SCB = sb.tile([128, 20], dtype=f32)
            nc.vector.tensor_copy(SCB[:], psb[:])
            m2U = sb.tile([128, 1], dtype=f32)
            tsm(m2U[:], SCB[:, 18:19], -2.0)
            p2V = sb.tile([128, 1], dtype=f32)
            tsm(p2V[:], SCB[:, 19:20], 2.0)
            m4U = sb.tile([128, 1], dtype=f32)
            tsm(m4U[:], SCB[:, 18:19], -4.0)
            p4V = sb.tile([128, 1], dtype=f32)
            tsm(p4V[:], SCB[:, 19:20], 4.0)

            # ---- stage D: grid row 2 = -(U*c2g + V*s2g) ----
            w2a = sb.tile([1, 128], dtype=f32)
            nc.scalar.mul(w2a[:], c2gt[:], SC[:, 18:19])
            w2b = sb.tile([1, 128], dtype=f32)
            nc.scalar.mul(w2b[:], s2gt[:], SC[:, 19:20])
            tt(w2a[:], w2a[:], w2b[:], OP.add)
            tsm(w2a[:], w2a[:], -1.0)
            nc.gpsimd.dma_start(out=grid[0:1, :], in_=w2a[:])

            # ---- stage E: pixel path (front hoisted above stage A) ----
            # V_k = (Ck*ci + a_k) + (Sk*si + amb_k)  via ACT scale/bias APs
            V = [sb.tile(P19, dtype=f32, name=f"vk{_k}") for _k in range(K)]
            VA = [sb.tile(P19, dtype=f32, name=f"va{_k}") for _k in range(K)]
            VB = [sb.tile(P19, dtype=f32, name=f"vb{_k}") for _k in range(K)]
            eng = [nc.vector, nc.vector, nc.vector]
            for k in range(K):
                nc.scalar.activation(VA[k][:], ci[:], AF.Identity,
                                     bias=SCB[:, 0 + k:1 + k], scale=SCB[:, 3 + k:4 + k])
                nc.scalar.activation(VB[k][:], si[:], AF.Identity,
                                     bias=SCB[:, 9 + k:10 + k], scale=SCB[:, 6 + k:7 + k])
                eng[k].tensor_tensor(V[k][:], VA[k][:], VB[k][:], OP.add)
            mu = sb.tile(P19, dtype=f32)
            tt(mu[:], V[0][:], V[1][:], OP.add)
            tt(mu[:], mu[:], V[2][:], OP.add)
            tsm(mu[:], mu[:], float(nf32(1.0 / 3.0)))
            for k in range(K):
                eng[k].tensor_tensor(V[k][:], V[k][:], mu[:], OP.subtract)  # d_k
            vv = sb.tile(P19, dtype=f32)
            ta = sb.tile(P19, dtype=f32)
            tg3 = sb.tile(P19, dtype=f32)
            tt(vv[:], V[0][:], V[0][:], OP.mult)
            nc.vector.tensor_tensor(ta[:], V[1][:], V[1][:], OP.mult)
            nc.vector.tensor_tensor(tg3[:], V[2][:], V[2][:], OP.mult)
            tt(vv[:], vv[:], ta[:], OP.add)
            tt(vv[:], vv[:], tg3[:], OP.add)
            tsm(vv[:], vv[:], 0.5 * SC_UP)                 # 2^55, exact
            sdv = sb.tile(P19, dtype=f32)
            nc.scalar.activation(sdv[:], vv[:], AF.Sqrt)
            tsm(sdv[:], sdv[:], SC_DN)
            sdi = sb.tile(P19, dtype=f32)
            nc.vector.reciprocal(sdi[:], sdv[:])
            for k in range(K):
                eng[k].tensor_tensor(V[k][:], V[k][:], sdi[:], OP.mult)  # NB_k
            # P = HP * sum_k NB_k*cw_k ; Q = HQ * sum_k NB_k*sw_k
            su_ = sb.tile(P19, dtype=f32)
            sv_ = sb.tile(P19, dtype=f32)
            tgb = sb.tile(P19, dtype=f32)
            nc.scalar.mul(su_[:], V[0][:], SCB[:, 12:13])
            nc.scalar.mul(ta[:], V[1][:], SCB[:, 13:14])
            tt(su_[:], su_[:], ta[:], OP.add)
            nc.scalar.mul(ta[:], V[2][:], SCB[:, 14:15])
            tt(su_[:], su_[:], ta[:], OP.add)
            nc.scalar.mul(sv_[:], V[0][:], SCB[:, 15:16])
            nc.scalar.mul(tgb[:], V[1][:], SCB[:, 16:17])
            nc.vector.tensor_tensor(sv_[:], sv_[:], tgb[:], OP.add)
            nc.scalar.mul(tgb[:], V[2][:], SCB[:, 17:18])
            nc.vector.tensor_tensor(sv_[:], sv_[:], tgb[:], OP.add)
            Pp = sb.tile(P19, dtype=f32)
            tsm(Pp[:], su_[:], HP)
            Qq = sb.tile(P19, dtype=f32)
            nc.vector.tensor_scalar_mul(Qq[:], sv_[:], HQ)
            PQcat = sb.tile([128, 2 * NT], dtype=f32)
            tsm(PQcat[:, 0:NT], su_[:], HPn)               # -P
            nc.vector.tensor_scalar_mul(PQcat[:, NT:2 * NT], sv_[:], HQn)  # -Q

            # ---- transpose -> slab lhsT  L3 = [[-P],[-Q],[1]] ----
            ident = sb.tile([128, 128], dtype=f32)
            make_identity(nc, ident[:])
            pt = ps.tile([2 * NT, 128], dtype=f32)
            nc.tensor.transpose(pt[:], PQcat[:], ident[:])
            Pt = sb.tile([2 * NT, 128], dtype=f32)
            nc.vector.tensor_copy(Pt[:], pt[:])
            L3 = sb.tile([3, NT * 128], dtype=f32)
            nc.gpsimd.memset(L3[0:1, :], 1.0)
            nc.gpsimd.dma_start(out=L3[1:2, :], in_=Pt[0:NT, :])
            nc.gpsimd.dma_start(out=L3[2:3, :], in_=Pt[NT:2 * NT, :])

            # ---- coarse argmin over 128-point grid, per slab ----
            GF = sb.tile(P19, dtype=f32)
            mx = sb.tile([128, 8], dtype=f32)
            mi = sb.tile([128, 8], dtype=u32)
            for j in range(NT):
                nps = ps2.tile([128, 128], dtype=f32)
                nc.tensor.matmul(nps[:], L3[:, 128 * j:128 * (j + 1)], grid[:],
                                 start=True, stop=True)
                nc.vector.max(mx[:], nps[:])
                nc.vector.max_index(mi[:], mx[:], nps[:])
                nc.vector.tensor_copy(GF[:, j:j + 1], mi[:, 0:1].bitcast(i32))

            # ---- Newton refinement (3 clamped steps) ----
            u4g = sb.tile(P19, dtype=f32)
            tsm(u4g[:], GF[:], 4.0 / G)       # pow2, bit-identical to /G then x4
            cgg = sb.tile(P19, dtype=f32)
            sgg = sb.tile(P19, dtype=f32)
            _emit_cossin(nc, sb, P19, u4g[:], cgg[:], sgg[:])
            thg = sb.tile(P19, dtype=f32)
            tsm(thg[:], GF[:], SPACING)
            delta = sb.tile(P19, dtype=f32)
            nc.vector.memset(delta[:], 0.0)
            x2 = sb.tile(P19, dtype=f32)
            x4 = sb.tile(P19, dtype=f32)
            cd = sb.tile(P19, dtype=f32)
            sd_ = sb.tile(P19, dtype=f32)
            ct = sb.tile(P19, dtype=f32)
            st = sb.tile(P19, dtype=f32)
            c2t = sb.tile(P19, dtype=f32)
            s2t = sb.tile(P19, dtype=f32)
            d1 = sb.tile(P19, dtype=f32)
            d2 = sb.tile(P19, dtype=f32)
            rec = sb.tile(P19, dtype=f32)
            step = sb.tile(P19, dtype=f32)
            tb = sb.tile(P19, dtype=f32)
            tbg = sb.tile(P19, dtype=f32)
            gt_ = nc.vector.tensor_tensor
            gs_ = nc.vector.tensor_scalar
            for it in range(3):
                # |delta| <= 0.0492 so x2 <= 2.5e-3: quartic terms < 3e-7,
                # below the f32 noise floor of this decode — drop them.
                tt(x2[:], delta[:], delta[:], OP.mult)
                ts(cd[:], x2[:], C2, 1.0, OP.mult, OP.add)         # 1 - x2/2
                gs_(sd_[:], x2[:], S3, 1.0, OP.mult, OP.add)       # 1 - x2/6
                gt_(sd_[:], sd_[:], delta[:], OP.mult)
                tt(ct[:], cgg[:], cd[:], OP.mult)
                gt_(tbg[:], sgg[:], sd_[:], OP.mult)
                tt(ct[:], ct[:], tbg[:], OP.subtract)
                gt_(st[:], sgg[:], cd[:], OP.mult)
                gt_(tbg[:], cgg[:], sd_[:], OP.mult)
                gt_(st[:], st[:], tbg[:], OP.add)
                tt(c2t[:], ct[:], ct[:], OP.mult)
                ts(c2t[:], c2t[:], 2.0, -1.0, OP.mult, OP.add)
                gt_(s2t[:], st[:], ct[:], OP.mult)
                nc.vector.tensor_scalar_mul(s2t[:], s2t[:], 2.0)
                # d1 = -2U*s2t + 2V*c2t - P*st + Q*ct   (vector)
                nc.scalar.mul(d1[:], s2t[:], m2U[:])
                nc.scalar.mul(tb[:], c2t[:], p2V[:])
                tt(d1[:], d1[:], tb[:], OP.add)
                tt(tb[:], Pp[:], st[:], OP.mult)
                tt(d1[:], d1[:], tb[:], OP.subtract)
                tt(tb[:], Qq[:], ct[:], OP.mult)
                tt(d1[:], d1[:], tb[:], OP.add)
                # d2 = -4U*c2t - 4V*s2t - P*ct - Q*st   (gpsimd)
                nc.scalar.mul(d2[:], c2t[:], m4U[:])
                nc.scalar.mul(tbg[:], s2t[:], p4V[:])
                gt_(d2[:], d2[:], tbg[:], OP.subtract)
                gt_(tbg[:], Pp[:], ct[:], OP.mult)
                gt_(d2[:], d2[:], tbg[:], OP.subtract)
                gt_(tbg[:], Qq[:], st[:], OP.mult)
                gt_(d2[:], d2[:], tbg[:], OP.subtract)
                gs_(d2[:], d2[:], 1e-6, None, OP.max)
                nc.vector.reciprocal(rec[:], d2[:])
                tt(step[:], d1[:], rec[:], OP.mult)
                ts(step[:], step[:], -SPACING, SPACING, OP.max, OP.min)
                tt(delta[:], delta[:], step[:], OP.subtract)

            # ---- n* = fold(floor((thg+delta)*N/2pi + 0.5)) ----
            nfv = sb.tile(P19, dtype=f32)
            tt(nfv[:], thg[:], delta[:], OP.add)
            tsm(nfv[:], nfv[:], NFC)
            tsa(nfv[:], nfv[:], 0.5)
            nst = sb.tile(P19, dtype=f32)
            _emit_floor(nc, sb, P19, nst[:], nfv[:])
            mlo = sb.tile(P19, dtype=f32)
            ts(mlo[:], nst[:], 0.0, None, OP.is_lt)
            tsm(mlo[:], mlo[:], float(N))
            tt(nst[:], nst[:], mlo[:], OP.add)
            mhi = sb.tile(P19, dtype=f32)
            ts(mhi[:], nst[:], float(N) - 0.5, None, OP.is_gt)
            tsm(mhi[:], mhi[:], float(N))
            tt(nst[:], nst[:], mhi[:], OP.subtract)
            nc.gpsimd.dma_start(out=OUT[:], in_=nst[:])
    return nc


_NC_CACHE = None


def _get_nc():
    global _NC_CACHE
    if _NC_CACHE is None:
        _NC_CACHE = _build()
    return _NC_CACHE


def _host_consts():
    m = np.arange(N, dtype=np.float64)
    th = 2.0 * np.pi * m / N
    cb = np.cos(th).astype(np.float32).reshape(125, 80)
    sbv = np.sin(th).astype(np.float32).reshape(125, 80)
    CB = np.repeat(cb[:, :, None], 3, axis=2).reshape(125, 240)
    SB = np.repeat(sbv[:, :, None], 3, axis=2).reshape(125, 240)
    g = np.arange(G, dtype=np.float64)
    tg = 2.0 * np.pi * g / G
    GRIDC = np.stack([np.cos(tg), np.sin(tg)]).astype(np.float32)
    C2Gv = np.cos(2.0 * tg).astype(np.float32)[None, :]
    S2Gv = np.sin(2.0 * tg).astype(np.float32)[None, :]
    return CB, SB, GRIDC, C2Gv, S2Gv


def kernel(gt_depths: np.ndarray, ModFs: np.ndarray, DemodFs: np.ndarray) -> np.ndarray:
    nc = _get_nc()
    CB, SB, GRIDC, C2Gv, S2Gv = _host_consts()
    MODRh = np.ascontiguousarray(ModFs, dtype=np.float32).reshape(125, 240)
    DEMRh = np.ascontiguousarray(DemodFs, dtype=np.float32).reshape(125, 240)
    flat = np.asarray(gt_depths, dtype=np.float32).reshape(-1)
    per = flat.reshape(NCORES, PPC)
    full = np.concatenate([per, np.zeros((NCORES, NT * 128 - PPC), np.float32)], axis=1)
    gins = full.reshape(NCORES, NT, 128).transpose(0, 2, 1)   # [8,128,19]
    ones125 = np.ones((125, 1), np.float32)
    ones1 = np.ones((1, 128), np.float32)
    ins = []
    for c in range(NCORES):
        ins.append({
            "GIN": np.ascontiguousarray(gins[c]),
            "MODR": MODRh, "DEMR": DEMRh, "CBD": CB, "SBD": SB,
            "ONES125": ones125, "ONES1": ones1,
            "GRIDC": GRIDC, "C2G": C2Gv, "S2G": S2Gv,
        })
    res = run_bass_kernel_spmd(nc, ins, core_ids=list(range(NCORES)))
    outs = np.stack([np.asarray(res.results[c]["OUT"]) for c in range(NCORES)])
    out = outs.transpose(0, 2, 1).reshape(NCORES, NT * 128)[:, :PPC].reshape(-1)
    return out.reshape(gt_depths.shape).astype(np.float32)

